# revision 12
# baseline (speedup 1.0000x reference)
# Swin-style window attention (B=256 windows, N=196, C=768, H=12) on 8 trn2 cores.
# Data-parallel over windows: 32 windows/core. Per core:
#   qT/kT = Wqk(bf16, stationary) @ x.T(bf16, moving)      [o, t] layout
#   V     = x.T(bf16, stationary) @ Wv(bf16); +vb fused into the PSUM->SBUF
#           copy (scalar_tensor_tensor with host-prebroadcast vb); V stored
#           strided [12 x (64 v-cols + ones-col + pad)] so each head's AV
#           stationary slice [m, 65] carries a built-in denominator column.
#   per (window, head):
#     S.T  = kT_h.T @ qT_h (bf16, K=64), issue order alternates the two
#            row-halves of a head pair so LDWEIGHTS pulls ahead of MATMULs
#     p    = exp(0.125*S.T) * exp(rpb).T      (one ACT + one DVE op, [128,392])
#     OT   = [V_h | ones].T @ p               (one matmul per key-block:
#                                              O rows 0:64, denominator row 64)
#     otn  = OT * recip(den) with recip broadcast across partitions by
#            gpsimd.partition_broadcast (no PE involvement), bf16 TT at 2x
#   yT = Wp(bf16, stationary) @ O.T(bf16) + pb
# Zero on-device transposes; all contractions land on partitions naturally.
import sys

sys.path.insert(0, "/opt/trn_rl_repo")

from contextlib import ExitStack

import ml_dtypes
import numpy as np

import concourse.bass as bass
import concourse.bacc as bacc
import concourse.mybir as mybir
import concourse.tile as tile
from concourse.bass_utils import run_bass_kernel_spmd

F32 = mybir.dt.float32
BF16 = mybir.dt.bfloat16
AF = mybir.ActivationFunctionType
ALU = mybir.AluOpType

_NC_CACHE = {}
NCORES = 8
B, N, C, H = 256, 196, 768, 12
HD = C // H  # 64
WPC = B // NCORES  # 32 windows per core
T = WPC * N  # 6272 tokens per core
CB = C // 128  # 6 contraction blocks
CHUNK_W = 2  # windows per chunk
VG = HD + 2  # 66: v col group stride (64 v cols + ones col + pad)
VW = H * VG  # 792: strided V tile width


def _install_ntff_hook():
    """Recreate the antenv.axon_hooks shim so trace=True works under axon."""
    import types

    if "antenv.axon_hooks" in sys.modules:
        return
    mod = types.ModuleType("antenv.axon_hooks")
    mod._hook = None
    mod.set_axon_ntff_profile_hook = lambda h: setattr(mod, "_hook", h)
    mod.get_axon_ntff_profile_hook = lambda: mod._hook
    sys.modules["antenv.axon_hooks"] = mod
    try:
        sys.path.insert(0, "/root/.axon_site/trn_agent_boot")
        from trn_boot import _ntff_profile_via_ctypes

        hook = _ntff_profile_via_ctypes("/opt/axon/libaxon_pjrt.so")
        if hook is not None:
            mod._hook = hook
    except Exception:
        pass


def _build_nc(wpc=WPC, chunk_w=CHUNK_W):
    t_total = wpc * N
    nchunk = wpc // chunk_w
    chunk_t = chunk_w * N

    nc = bacc.Bacc("TRN2", target_bir_lowering=False, debug=False,
                   num_devices=NCORES)
    xT_d = nc.dram_tensor("xT", [C, t_total], BF16, kind="ExternalInput").ap()
    wqk_d = nc.dram_tensor("wqkT", [C, 2 * C], BF16, kind="ExternalInput").ap()
    wv_d = nc.dram_tensor("wvT", [C, C], BF16, kind="ExternalInput").ap()
    wp_d = nc.dram_tensor("projwT", [C, C], BF16, kind="ExternalInput").ap()
    qb_d = nc.dram_tensor("qbT", [128, CB], F32, kind="ExternalInput").ap()
    pb_d = nc.dram_tensor("pbT", [128, CB], F32, kind="ExternalInput").ap()
    erp_d = nc.dram_tensor("erpT", [H, 128, 2 * N], BF16,
                           kind="ExternalInput").ap()
    rb_scr = nc.dram_tensor("rb_scr", [2, H * N], BF16, kind="Internal").ap()
    yT_d = nc.dram_tensor("yT", [C, t_total], F32, kind="ExternalOutput").ap()

    with tile.TileContext(nc) as tc, ExitStack() as ctx:
        const = ctx.enter_context(tc.tile_pool(name="const", bufs=1))
        wpool = ctx.enter_context(tc.tile_pool(name="w", bufs=1))
        xpool = ctx.enter_context(tc.tile_pool(name="x", bufs=2))
        qkpool = ctx.enter_context(tc.tile_pool(name="qk", bufs=2))
        vpool = ctx.enter_context(tc.tile_pool(name="v", bufs=2))
        otpool = ctx.enter_context(tc.tile_pool(name="ot", bufs=2))
        ppool = ctx.enter_context(tc.tile_pool(name="p", bufs=6))
        rpool = ctx.enter_context(tc.tile_pool(name="r", bufs=2))
        bpool = ctx.enter_context(tc.tile_pool(name="bb", bufs=2))
        opool = ctx.enter_context(tc.tile_pool(name="ou", bufs=2))
        ypool = ctx.enter_context(tc.tile_pool(name="y", bufs=2))
        ps_mm = ctx.enter_context(tc.tile_pool(name="psmm", bufs=2,
                                               space="PSUM"))
        ps_st = ctx.enter_context(tc.tile_pool(name="psst", bufs=4,
                                               space="PSUM"))
        ps_ot = ctx.enter_context(tc.tile_pool(name="psot", bufs=2,
                                               space="PSUM"))

        # ---- resident constants / weights ----
        wqk, wv, wp = [], [], []
        for cb in range(CB):
            t = wpool.tile([128, 2 * C], BF16, tag=f"wqk{cb}")
            nc.sync.dma_start(t[:], wqk_d[cb * 128:(cb + 1) * 128, :])
            wqk.append(t)
            t = wpool.tile([128, C], BF16, tag=f"wv{cb}")
            nc.sync.dma_start(t[:], wv_d[cb * 128:(cb + 1) * 128, :])
            wv.append(t)
            t = wpool.tile([128, C], BF16, tag=f"wp{cb}")
            nc.sync.dma_start(t[:], wp_d[cb * 128:(cb + 1) * 128, :])
            wp.append(t)
        erp = []
        for h in range(H):
            t = wpool.tile([128, 2 * N], BF16, tag=f"erp{h}")
            nc.sync.dma_start(t[:], erp_d[h, :, :])
            erp.append(t)
        qb = const.tile([128, CB], F32)
        nc.sync.dma_start(qb[:], qb_d[:, :])
        pb = const.tile([128, CB], F32)
        nc.sync.dma_start(pb[:], pb_d[:, :])

        fin_pending = None
        proj_pending = None

        for ch in range(nchunk):
            t0 = ch * chunk_t
            xt = []
            for cb in range(CB):
                t = xpool.tile([128, chunk_t], BF16, tag=f"xt{cb}")
                nc.sync.dma_start(t[:], xT_d[cb * 128:(cb + 1) * 128,
                                             t0:t0 + chunk_t])
                xt.append(t)

            tslices = [(i * 512, min(512, chunk_t - i * 512))
                       for i in range((chunk_t + 511) // 512)]

            # ---- Q.T / K.T ----
            qT, kT = [], []
            for ob in range(CB):
                t = qkpool.tile([128, chunk_t], BF16, tag=f"qT{ob}")
                qT.append(t)
            for ob in range(CB):
                t = qkpool.tile([128, chunk_t + 64], BF16, tag=f"kT{ob}")
                nc.vector.memset(t[:, chunk_t:chunk_t + 64], 0.0)
                kT.append(t)
            for ob in range(2 * CB):
                dst = qT[ob] if ob < CB else kT[ob - CB]
                o = ob * 128
                pt = ps_mm.tile([128, chunk_t], F32, tag="mm")
                for cb in range(CB):
                    for (ts, tl) in tslices:
                        nc.tensor.matmul(
                            pt[:, ts:ts + tl],
                            wqk[cb][:, o:o + 128],
                            xt[cb][:, ts:ts + tl],
                            start=(cb == 0), stop=(cb == CB - 1))
                if ob < CB:  # q: bias here, softmax scale folded into exp
                    nc.scalar.activation(dst[:], pt[:, 0:chunk_t], AF.Identity,
                                         bias=qb[:, ob:ob + 1])
                else:  # k: plain copy/cast
                    nc.scalar.copy(dst[:, 0:chunk_t], pt[:, 0:chunk_t])

            if fin_pending is not None:
                finish_window(*fin_pending)
                fin_pending = None
            if proj_pending is not None:
                proj_pending()
                proj_pending = None

            # ---- V token-major, strided per head with ones column ----
            # vt layout: head h occupies cols VG*h .. VG*h+63 (v), col
            # VG*h+64 is 1.0 (denominator ones), col VG*h+65 unused.
            vtiles = []
            for w in range(chunk_w):
                wrow = []
                for (moff, mlen) in ((0, 128), (128, 68)):
                    trel = w * N + moff
                    vt = vpool.tile([128, VW], BF16, tag=f"vb{w}_{moff}")
                    # two 384-wide psum tiles (6 heads each); cast strided
                    # into head groups (v bias is folded into the proj bias
                    # on the host: softmax rows sum to 1)
                    for half in range(2):
                        noff = half * 384
                        pv = ps_mm.tile([128, chunk_t], F32, tag="mm")
                        for cb in range(CB):
                            nc.tensor.matmul(
                                pv[0:mlen, 0:384],
                                xt[cb][:, trel:trel + mlen],
                                wv[cb][:, noff:noff + 384],
                                start=(cb == 0), stop=(cb == CB - 1))
                        vt_base = vt[0:mlen, half * 6 * VG:VW]
                        vt_str = bass.AP(vt_base.tensor, vt_base.offset,
                                         [vt_base.ap[0], [VG, 6], [1, HD]])
                        nc.vector.tensor_copy(vt_str, pv[0:mlen, 0:384])
                    ones_base = vt[0:mlen, HD:HD + 1]
                    ones_str = bass.AP(ones_base.tensor, ones_base.offset,
                                       [ones_base.ap[0], [VG, H], [1, 1]])
                    nc.vector.memset(ones_str, 1.0)
                    wrow.append(vt)
                vtiles.append(wrow)

            # ---- attention per (window, head) ----
            ot_sb = []
            for ob in range(CB):
                t = otpool.tile([128, chunk_t], BF16, tag=f"ot{ob}")
                ot_sb.append(t)

            def finish_window(wq_tok, otu, rbb, ot_sb):
                for h in range(H):
                    ob = h // 2
                    prt = (h % 2) * 64
                    nc.vector.tensor_mul(
                        ot_sb[ob][prt:prt + 64, wq_tok:wq_tok + N],
                        otu[0:64, h * N:(h + 1) * N],
                        rbb[0:64, h * N:(h + 1) * N])

            for w in range(chunk_w):
                wq_tok = w * N
                otu = opool.tile([65, H * N], BF16, tag="otun")
                for hp in range(H // 2):
                    h0, h1 = 2 * hp, 2 * hp + 1
                    ob = h0 // 2
                    # S matmuls: alternate row halves so each LDWEIGHTS
                    # targets the row group the previous MATMUL is not using
                    st0 = ps_st.tile([128, 2 * N], F32, tag="st")
                    st1 = ps_st.tile([128, 2 * N], F32, tag="st")
                    sts = (st0, st1)
                    qh = (qT[ob][0:64, wq_tok:wq_tok + N],
                          qT[ob][64:128, wq_tok:wq_tok + N])
                    for blk in range(2):
                        c0 = wq_tok + blk * 128
                        for hi in range(2):
                            prt = hi * 64
                            nc.tensor.matmul(
                                sts[hi][:, blk * N:(blk + 1) * N],
                                kT[ob][prt:prt + 64, c0:c0 + 128],
                                qh[hi], start=True, stop=True)
                    ps_ = []
                    for hi, h in enumerate((h0, h1)):
                        p = ppool.tile([128, 2 * N], BF16, tag="p")
                        nc.scalar.activation(p[:], sts[hi][:], AF.Exp,
                                             scale=0.125)
                        nc.vector.tensor_mul(p[:], p[:], erp[h][:])
                        ps_.append(p)
                    # pair-packed AV output: h0 -> cols 0:N, h1 -> N:2N of
                    # one PSUM bank tile, copied out with a single CAST
                    ot = ps_ot.tile([65, 2 * N], F32, tag="ot")
                    for hi, h in enumerate((h0, h1)):
                        p = ps_[hi]
                        for bi, (moff, mlen) in enumerate(((0, 128),
                                                          (128, 68))):
                            nc.tensor.matmul(
                                ot[:, hi * N:(hi + 1) * N],
                                vtiles[w][bi][0:mlen, VG * h:VG * h + 65],
                                p[0:mlen, bi * N:(bi + 1) * N],
                                start=(bi == 0), stop=(bi == 1))
                    nc.vector.tensor_copy(
                        otu[:, h0 * N:(h0 + 2) * N], ot[:])
                srcrow = otu[64:65, 0:H * N]
                den = rpool.tile([H, N], BF16, tag="den")
                nc.gpsimd.dma_start(
                    den[:], bass.AP(srcrow.tensor, srcrow.offset,
                                    [srcrow.ap[0], [N, H], [1, N]]))
                denf = rpool.tile([H, N], F32, tag="denf")
                nc.vector.tensor_copy(denf[:], den[:])
                rec = rpool.tile([H, N], F32, tag="rec")
                nc.vector.reciprocal_approx_fast(rec[:], denf[:])
                recb = rpool.tile([H, N], BF16, tag="recb")
                nc.vector.tensor_copy(recb[:], rec[:])
                # broadcast 1/den across partitions via a DRAM bounce:
                # write [H,N] linear, read back with 0-stride partition dim
                scr = rb_scr[w % 2, :]
                nc.sync.dma_start(scr, recb[:, :])
                rbb = bpool.tile([64, H * N], BF16, tag="rbb")
                nc.sync.dma_start(
                    rbb[:], bass.AP(scr.tensor, scr.offset,
                                    [[0, 64], [1, H * N]]))
                if fin_pending is not None:
                    finish_window(*fin_pending)
                fin_pending = (wq_tok, otu, rbb, ot_sb)

            # ---- proj (deferred one chunk) ----
            def make_proj(t0, ot_sb, yts):
                def emit_proj():
                    for opb in range(CB):
                        o = opb * 128
                        pt = ps_mm.tile([128, chunk_t], F32, tag="mm")
                        for (ts, tl) in yts:
                            for ob in range(CB):
                                nc.tensor.matmul(
                                    pt[:, ts:ts + tl],
                                    wp[ob][:, o:o + 128],
                                    ot_sb[ob][:, ts:ts + tl],
                                    start=(ob == 0), stop=(ob == CB - 1))
                        yt = ypool.tile([128, chunk_t], F32, tag="y")
                        nc.scalar.activation(yt[:], pt[:, 0:chunk_t],
                                             AF.Identity,
                                             bias=pb[:, opb:opb + 1])
                        nc.sync.dma_start(yT_d[o:o + 128, t0:t0 + chunk_t],
                                          yt[:])
                return emit_proj
            proj_pending = make_proj(t0, ot_sb, tslices)

        if fin_pending is not None:
            finish_window(*fin_pending)
        if proj_pending is not None:
            proj_pending()

    nc.compile()
    return nc


def _host_prep(x, qkv_w, q_bias, v_bias, rpb_table, proj_w, proj_b, rel_index,
               wpc=WPC):
    x = np.asarray(x, np.float32)
    ncores = x.shape[0] // wpc
    t_total = wpc * N
    xT = np.ascontiguousarray(
        x.reshape(ncores, t_total, C).transpose(0, 2, 1)).astype(
            ml_dtypes.bfloat16)
    qkv_w = np.asarray(qkv_w, np.float32)
    wqkT = np.ascontiguousarray(qkv_w[0:2 * C].T).astype(ml_dtypes.bfloat16)
    wvT = np.ascontiguousarray(qkv_w[2 * C:3 * C].T).astype(
        ml_dtypes.bfloat16)
    projwT = np.ascontiguousarray(
        np.asarray(proj_w, np.float32).T).astype(ml_dtypes.bfloat16)
    qbT = np.ascontiguousarray(
        np.asarray(q_bias, np.float32).reshape(CB, 128).T)
    # softmax rows sum to 1, so the v bias contributes proj_w @ v_bias to
    # every output token: fold it into the proj bias
    pb_eff = (np.asarray(proj_b, np.float32)
              + np.asarray(proj_w, np.float32) @ np.asarray(v_bias,
                                                            np.float32))
    pbT = np.ascontiguousarray(pb_eff.reshape(CB, 128).T)
    rel = np.asarray(rel_index).reshape(N, N)
    rpb = np.asarray(rpb_table, np.float32)[rel]              # [n, m, H]
    erp_full = np.exp(rpb).transpose(2, 1, 0)                 # [H, m, n]
    erpT = np.zeros((H, 128, 2 * N), np.float32)
    erpT[:, :, :N] = erp_full[:, 0:128, :]
    erpT[:, 0:68, N:] = erp_full[:, 128:196, :]
    erpT = erpT.astype(ml_dtypes.bfloat16)
    return xT, wqkT, wvT, projwT, qbT, pbT, erpT


def kernel(x, qkv_w, q_bias, v_bias, rpb_table, proj_w, proj_b, rel_index,
           num_heads=12, _trace=False):
    xT, wqkT, wvT, projwT, qbT, pbT, erpT = _host_prep(
        x, qkv_w, q_bias, v_bias, rpb_table, proj_w, proj_b, rel_index)
    if _trace:
        _install_ntff_hook()
    nc = _NC_CACHE.get("nc")
    if nc is None:
        nc = _build_nc()
        _NC_CACHE["nc"] = nc
    in_maps = [
        {"xT": np.ascontiguousarray(xT[c]), "wqkT": wqkT, "wvT": wvT,
         "projwT": projwT, "qbT": qbT, "pbT": pbT, "erpT": erpT}
        for c in range(NCORES)
    ]
    res = run_bass_kernel_spmd(nc, in_maps, core_ids=list(range(NCORES)),
                               trace=_trace)
    yT = np.stack([res.results[c]["yT"] for c in range(NCORES)])
    out = np.ascontiguousarray(yT.transpose(0, 2, 1)).reshape(B, N, C)
    if _trace:
        kernel._last_exec_time_ns = res.exec_time_ns
        kernel._last_results = res
    return out.astype(np.float32)


# revision 13
# speedup vs baseline: 1.0921x; 1.0921x over previous
# Swin-style window attention (B=256 windows, N=196, C=768, H=12) on 8 trn2 cores.
# Data-parallel over windows: 32 windows/core. Per core:
#   qT/kT = Wqk(bf16, stationary) @ x.T(bf16, moving)      [o, t] layout
#   V     = x.T(bf16, stationary) @ Wv(bf16); +vb fused into the PSUM->SBUF
#           copy (scalar_tensor_tensor with host-prebroadcast vb); V stored
#           strided [12 x (64 v-cols + ones-col + pad)] so each head's AV
#           stationary slice [m, 65] carries a built-in denominator column.
#   per (window, head):
#     S.T  = kT_h.T @ qT_h (bf16, K=64), issue order alternates the two
#            row-halves of a head pair so LDWEIGHTS pulls ahead of MATMULs
#     p    = exp(0.125*S.T) * exp(rpb).T      (one ACT + one DVE op, [128,392])
#     OT   = [V_h | ones].T @ p               (one matmul per key-block:
#                                              O rows 0:64, denominator row 64)
#     otn  = OT * recip(den) with recip broadcast across partitions by
#            gpsimd.partition_broadcast (no PE involvement), bf16 TT at 2x
#   yT = Wp(bf16, stationary) @ O.T(bf16) + pb
# Zero on-device transposes; all contractions land on partitions naturally.
import sys

sys.path.insert(0, "/opt/trn_rl_repo")

from contextlib import ExitStack

import ml_dtypes
import numpy as np

import concourse.bass as bass
import concourse.bacc as bacc
import concourse.mybir as mybir
import concourse.tile as tile
from concourse.bass_utils import run_bass_kernel_spmd

F32 = mybir.dt.float32
BF16 = mybir.dt.bfloat16
AF = mybir.ActivationFunctionType
ALU = mybir.AluOpType

_NC_CACHE = {}
NCORES = 8
B, N, C, H = 256, 196, 768, 12
HD = C // H  # 64
WPC = B // NCORES  # 32 windows per core
T = WPC * N  # 6272 tokens per core
CB = C // 128  # 6 contraction blocks
CHUNK_W = 4  # windows per chunk
VG = HD + 2  # 66: v col group stride (64 v cols + ones col + pad)
VW = H * VG  # 792: strided V tile width


def _install_ntff_hook():
    """Recreate the antenv.axon_hooks shim so trace=True works under axon."""
    import types

    if "antenv.axon_hooks" in sys.modules:
        return
    mod = types.ModuleType("antenv.axon_hooks")
    mod._hook = None
    mod.set_axon_ntff_profile_hook = lambda h: setattr(mod, "_hook", h)
    mod.get_axon_ntff_profile_hook = lambda: mod._hook
    sys.modules["antenv.axon_hooks"] = mod
    try:
        sys.path.insert(0, "/root/.axon_site/trn_agent_boot")
        from trn_boot import _ntff_profile_via_ctypes

        hook = _ntff_profile_via_ctypes("/opt/axon/libaxon_pjrt.so")
        if hook is not None:
            mod._hook = hook
    except Exception:
        pass


def _build_nc(wpc=WPC, chunk_w=CHUNK_W):
    t_total = wpc * N
    nchunk = wpc // chunk_w
    chunk_t = chunk_w * N

    nc = bacc.Bacc("TRN2", target_bir_lowering=False, debug=False,
                   num_devices=NCORES)
    xT_d = nc.dram_tensor("xT", [C, t_total], BF16, kind="ExternalInput").ap()
    wqk_d = nc.dram_tensor("wqkT", [C, 2 * C], BF16, kind="ExternalInput").ap()
    wv_d = nc.dram_tensor("wvT", [C, C], BF16, kind="ExternalInput").ap()
    wp_d = nc.dram_tensor("projwT", [C, C], BF16, kind="ExternalInput").ap()
    qb_d = nc.dram_tensor("qbT", [128, CB], F32, kind="ExternalInput").ap()
    pb_d = nc.dram_tensor("pbT", [128, CB], F32, kind="ExternalInput").ap()
    erp_d = nc.dram_tensor("erpT", [H // 2, 128, 904], BF16,
                           kind="ExternalInput").ap()
    rb_scr = nc.dram_tensor("rb_scr", [2, H * N], BF16, kind="Internal").ap()
    yT_d = nc.dram_tensor("yT", [C, t_total], F32, kind="ExternalOutput").ap()

    with tile.TileContext(nc) as tc, ExitStack() as ctx:
        const = ctx.enter_context(tc.tile_pool(name="const", bufs=1))
        wpool = ctx.enter_context(tc.tile_pool(name="w", bufs=1))
        xpool = ctx.enter_context(tc.tile_pool(name="x", bufs=2))
        qkpool = ctx.enter_context(tc.tile_pool(name="qk", bufs=2))
        vpool = ctx.enter_context(tc.tile_pool(name="v", bufs=2))
        otpool = ctx.enter_context(tc.tile_pool(name="ot", bufs=2))
        ppool = ctx.enter_context(tc.tile_pool(name="p", bufs=6))
        rpool = ctx.enter_context(tc.tile_pool(name="r", bufs=2))
        bpool = ctx.enter_context(tc.tile_pool(name="bb", bufs=2))
        opool = ctx.enter_context(tc.tile_pool(name="ou", bufs=2))
        ypool = ctx.enter_context(tc.tile_pool(name="y", bufs=2))
        ps_mm = ctx.enter_context(tc.tile_pool(name="psmm", bufs=2,
                                               space="PSUM"))
        ps_st = ctx.enter_context(tc.tile_pool(name="psst", bufs=2,
                                               space="PSUM"))
        ps_ot = ctx.enter_context(tc.tile_pool(name="psot", bufs=2,
                                               space="PSUM"))

        # ---- resident constants / weights ----
        wqk, wv, wp = [], [], []
        for cb in range(CB):
            t = wpool.tile([128, 2 * C], BF16, tag=f"wqk{cb}")
            nc.sync.dma_start(t[:], wqk_d[cb * 128:(cb + 1) * 128, :])
            wqk.append(t)
            t = wpool.tile([128, C], BF16, tag=f"wv{cb}")
            nc.sync.dma_start(t[:], wv_d[cb * 128:(cb + 1) * 128, :])
            wv.append(t)
            t = wpool.tile([128, C], BF16, tag=f"wp{cb}")
            nc.sync.dma_start(t[:], wp_d[cb * 128:(cb + 1) * 128, :])
            wp.append(t)
        erp = []
        for hp in range(H // 2):
            t = wpool.tile([128, 904], BF16, tag=f"erp{hp}")
            nc.sync.dma_start(t[:], erp_d[hp, :, :])
            erp.append(t)
        qb = const.tile([128, CB], F32)
        nc.sync.dma_start(qb[:], qb_d[:, :])
        pb = const.tile([128, CB], F32)
        nc.sync.dma_start(pb[:], pb_d[:, :])

        fin_pending = None
        proj_pending = None

        for ch in range(nchunk):
            t0 = ch * chunk_t
            xt = []
            for cb in range(CB):
                t = xpool.tile([128, chunk_t], BF16, tag=f"xt{cb}")
                nc.sync.dma_start(t[:], xT_d[cb * 128:(cb + 1) * 128,
                                             t0:t0 + chunk_t])
                xt.append(t)

            tslices = [(i * 512, min(512, chunk_t - i * 512))
                       for i in range((chunk_t + 511) // 512)]

            # ---- Q.T / K.T ----
            qT, kT = [], []
            for ob in range(CB):
                t = qkpool.tile([128, chunk_t], BF16, tag=f"qT{ob}")
                qT.append(t)
            for ob in range(CB):
                t = qkpool.tile([128, chunk_t + 64], BF16, tag=f"kT{ob}")
                nc.vector.memset(t[:, chunk_t:chunk_t + 64], 0.0)
                kT.append(t)
            for ob in range(2 * CB):
                dst = qT[ob] if ob < CB else kT[ob - CB]
                o = ob * 128
                for (ts, tl) in tslices:
                    pt = ps_mm.tile([128, 512], F32, tag="mm")
                    for cb in range(CB):
                        nc.tensor.matmul(
                            pt[:, 0:tl],
                            wqk[cb][:, o:o + 128],
                            xt[cb][:, ts:ts + tl],
                            start=(cb == 0), stop=(cb == CB - 1))
                    if ob < CB:  # q: bias here, scale folded into exp
                        nc.scalar.activation(dst[:, ts:ts + tl], pt[:, 0:tl],
                                             AF.Identity,
                                             bias=qb[:, ob:ob + 1])
                    else:  # k: plain copy/cast
                        nc.scalar.copy(dst[:, ts:ts + tl], pt[:, 0:tl])

            if fin_pending is not None:
                finish_window(*fin_pending)
                fin_pending = None
            if proj_pending is not None:
                proj_pending()
                proj_pending = None

            # ---- V token-major, strided per head with ones column ----
            # vt layout: head h occupies cols VG*h .. VG*h+63 (v), col
            # VG*h+64 is 1.0 (denominator ones), col VG*h+65 unused.
            vtiles = []
            for w in range(chunk_w):
                wrow = []
                for (moff, mlen) in ((0, 128), (128, 68)):
                    trel = w * N + moff
                    vt = vpool.tile([128, VW], BF16, tag=f"vb{w}_{moff}")
                    # two 384-wide psum tiles (6 heads each); cast strided
                    # into head groups (v bias is folded into the proj bias
                    # on the host: softmax rows sum to 1)
                    for half in range(2):
                        noff = half * 384
                        pv = ps_mm.tile([128, 512], F32, tag="mm")
                        for cb in range(CB):
                            nc.tensor.matmul(
                                pv[0:mlen, 0:384],
                                xt[cb][:, trel:trel + mlen],
                                wv[cb][:, noff:noff + 384],
                                start=(cb == 0), stop=(cb == CB - 1))
                        vt_base = vt[0:mlen, half * 6 * VG:VW]
                        vt_str = bass.AP(vt_base.tensor, vt_base.offset,
                                         [vt_base.ap[0], [VG, 6], [1, HD]])
                        nc.vector.tensor_copy(vt_str, pv[0:mlen, 0:384])
                    ones_base = vt[0:mlen, HD:HD + 1]
                    ones_str = bass.AP(ones_base.tensor, ones_base.offset,
                                       [ones_base.ap[0], [VG, H], [1, 1]])
                    nc.vector.memset(ones_str, 1.0)
                    wrow.append(vt)
                vtiles.append(wrow)

            # ---- attention per (window, head) ----
            ot_sb = []
            for ob in range(CB):
                t = otpool.tile([128, chunk_t], BF16, tag=f"ot{ob}")
                ot_sb.append(t)

            def finish_window(wq_tok, otu, rbb, ot_sb):
                for h in range(H):
                    ob = h // 2
                    prt = (h % 2) * 64
                    nc.vector.tensor_mul(
                        ot_sb[ob][prt:prt + 64, wq_tok:wq_tok + N],
                        otu[0:64, h * N:(h + 1) * N],
                        rbb[0:64, h * N:(h + 1) * N])

            for w in range(chunk_w):
                wq_tok = w * N
                otu = opool.tile([65, H * N], BF16, tag="otun")
                for hp in range(H // 2):
                    h0, h1 = 2 * hp, 2 * hp + 1
                    ob = h0 // 2
                    # paired S tile: h0 in cols 0:392 (bank 0), h1 in cols
                    # 512:904 (bank 1); alternate row halves so LDWEIGHTS
                    # targets the row group the previous MATMUL is not using
                    st = ps_st.tile([128, 1024], F32, tag="st")
                    qh = (qT[ob][0:64, wq_tok:wq_tok + N],
                          qT[ob][64:128, wq_tok:wq_tok + N])
                    for blk in range(2):
                        c0 = wq_tok + blk * 128
                        for hi in range(2):
                            prt = hi * 64
                            nc.tensor.matmul(
                                st[:, hi * 512 + blk * N:
                                    hi * 512 + (blk + 1) * N],
                                kT[ob][prt:prt + 64, c0:c0 + 128],
                                qh[hi], start=True, stop=True)
                    # one exp + one rpb multiply for the whole pair (the
                    # 392:512 gap is never read downstream)
                    p = ppool.tile([128, 904], BF16, tag="p")
                    nc.scalar.activation(p[:], st[:, 0:904], AF.Exp,
                                         scale=0.125)
                    nc.vector.tensor_mul(p[:], p[:], erp[hp][:])
                    # pair-packed AV output: h0 -> cols 0:N, h1 -> N:2N of
                    # one PSUM bank tile, copied out with a single CAST
                    ot = ps_ot.tile([65, 2 * N], F32, tag="ot")
                    for hi, h in enumerate((h0, h1)):
                        for bi, (moff, mlen) in enumerate(((0, 128),
                                                          (128, 68))):
                            nc.tensor.matmul(
                                ot[:, hi * N:(hi + 1) * N],
                                vtiles[w][bi][0:mlen, VG * h:VG * h + 65],
                                p[0:mlen, hi * 512 + bi * N:
                                  hi * 512 + (bi + 1) * N],
                                start=(bi == 0), stop=(bi == 1))
                    nc.vector.tensor_copy(
                        otu[:, h0 * N:(h0 + 2) * N], ot[:])
                srcrow = otu[64:65, 0:H * N]
                den = rpool.tile([H, N], BF16, tag="den")
                nc.gpsimd.dma_start(
                    den[:], bass.AP(srcrow.tensor, srcrow.offset,
                                    [srcrow.ap[0], [N, H], [1, N]]))
                denf = rpool.tile([H, N], F32, tag="denf")
                nc.vector.tensor_copy(denf[:], den[:])
                rec = rpool.tile([H, N], F32, tag="rec")
                nc.vector.reciprocal_approx_fast(rec[:], denf[:])
                recb = rpool.tile([H, N], BF16, tag="recb")
                nc.vector.tensor_copy(recb[:], rec[:])
                # broadcast 1/den across partitions via a DRAM bounce:
                # write [H,N] linear, read back with 0-stride partition dim
                scr = rb_scr[w % 2, :]
                nc.sync.dma_start(scr, recb[:, :])
                rbb = bpool.tile([64, H * N], BF16, tag="rbb")
                nc.sync.dma_start(
                    rbb[:], bass.AP(scr.tensor, scr.offset,
                                    [[0, 64], [1, H * N]]))
                if fin_pending is not None:
                    finish_window(*fin_pending)
                fin_pending = (wq_tok, otu, rbb, ot_sb)

            # ---- proj (deferred one chunk) ----
            def make_proj(t0, ot_sb, yts):
                def emit_proj():
                    for opb in range(CB):
                        o = opb * 128
                        yt = ypool.tile([128, chunk_t], F32, tag="y")
                        for (ts, tl) in yts:
                            pt = ps_mm.tile([128, 512], F32, tag="mm")
                            for ob in range(CB):
                                nc.tensor.matmul(
                                    pt[:, 0:tl],
                                    wp[ob][:, o:o + 128],
                                    ot_sb[ob][:, ts:ts + tl],
                                    start=(ob == 0), stop=(ob == CB - 1))
                            nc.scalar.activation(yt[:, ts:ts + tl],
                                                 pt[:, 0:tl],
                                                 AF.Identity,
                                                 bias=pb[:, opb:opb + 1])
                        nc.sync.dma_start(yT_d[o:o + 128, t0:t0 + chunk_t],
                                          yt[:])
                return emit_proj
            proj_pending = make_proj(t0, ot_sb, tslices)

        if fin_pending is not None:
            finish_window(*fin_pending)
        if proj_pending is not None:
            proj_pending()

    nc.compile()
    return nc


def _host_prep(x, qkv_w, q_bias, v_bias, rpb_table, proj_w, proj_b, rel_index,
               wpc=WPC):
    x = np.asarray(x, np.float32)
    ncores = x.shape[0] // wpc
    t_total = wpc * N
    xT = np.ascontiguousarray(
        x.reshape(ncores, t_total, C).transpose(0, 2, 1)).astype(
            ml_dtypes.bfloat16)
    qkv_w = np.asarray(qkv_w, np.float32)
    wqkT = np.ascontiguousarray(qkv_w[0:2 * C].T).astype(ml_dtypes.bfloat16)
    wvT = np.ascontiguousarray(qkv_w[2 * C:3 * C].T).astype(
        ml_dtypes.bfloat16)
    projwT = np.ascontiguousarray(
        np.asarray(proj_w, np.float32).T).astype(ml_dtypes.bfloat16)
    qbT = np.ascontiguousarray(
        np.asarray(q_bias, np.float32).reshape(CB, 128).T)
    # softmax rows sum to 1, so the v bias contributes proj_w @ v_bias to
    # every output token: fold it into the proj bias
    pb_eff = (np.asarray(proj_b, np.float32)
              + np.asarray(proj_w, np.float32) @ np.asarray(v_bias,
                                                            np.float32))
    pbT = np.ascontiguousarray(pb_eff.reshape(CB, 128).T)
    rel = np.asarray(rel_index).reshape(N, N)
    rpb = np.asarray(rpb_table, np.float32)[rel]              # [n, m, H]
    erp_full = np.exp(rpb).transpose(2, 1, 0)                 # [H, m, n]
    erpT = np.zeros((H // 2, 128, 904), np.float32)
    for hp in range(H // 2):
        for hi in range(2):
            o = hi * 512
            erpT[hp, :, o:o + N] = erp_full[2 * hp + hi, 0:128, :]
            erpT[hp, 0:68, o + N:o + 2 * N] = erp_full[2 * hp + hi,
                                                       128:196, :]
    erpT = erpT.astype(ml_dtypes.bfloat16)
    return xT, wqkT, wvT, projwT, qbT, pbT, erpT


def kernel(x, qkv_w, q_bias, v_bias, rpb_table, proj_w, proj_b, rel_index,
           num_heads=12, _trace=False):
    xT, wqkT, wvT, projwT, qbT, pbT, erpT = _host_prep(
        x, qkv_w, q_bias, v_bias, rpb_table, proj_w, proj_b, rel_index)
    if _trace:
        _install_ntff_hook()
    nc = _NC_CACHE.get("nc")
    if nc is None:
        nc = _build_nc()
        _NC_CACHE["nc"] = nc
    in_maps = [
        {"xT": np.ascontiguousarray(xT[c]), "wqkT": wqkT, "wvT": wvT,
         "projwT": projwT, "qbT": qbT, "pbT": pbT, "erpT": erpT}
        for c in range(NCORES)
    ]
    res = run_bass_kernel_spmd(nc, in_maps, core_ids=list(range(NCORES)),
                               trace=_trace)
    yT = np.stack([res.results[c]["yT"] for c in range(NCORES)])
    out = np.ascontiguousarray(yT.transpose(0, 2, 1)).reshape(B, N, C)
    if _trace:
        kernel._last_exec_time_ns = res.exec_time_ns
        kernel._last_results = res
    return out.astype(np.float32)
